# revision 11
# baseline (speedup 1.0000x reference)
"""Trainium2 Bass kernel for nn_BloodhoundSub_12463995093069.

2-layer decoder with broadcast cross-attention -> cosine similarity [8, 32].

Sharding: candidates (BC=32) split 4 per core across 8 cores. Each core runs
the full decoder for its 4 candidates against all 8 query batches; the host
concatenates the per-core [8, 4] outputs along axis 1.

Layout: feature-major activations (features on partitions, tokens free).
Q/K/V and FFN projections run in fp8-e4m3 DoubleRow mode (2x PE rate,
weights pre-scaled by 64 on the host, rescaled in the PSUM drain); the
residual stream x is bf16 with an fp8 shadow written during LN-apply.
Attention scores for all candidate sub-blocks of a head share one PSUM bank
so exp/denominator work runs on [128,512] slabs. Cross-partition reductions
(LN stats, softmax denominators, partition broadcasts) are selector matmuls
on the PE. PSUM drains round-robin between the scalar and gpsimd engines.
"""

import sys

if "/opt/trn_rl_repo" not in sys.path:
    sys.path.insert(0, "/opt/trn_rl_repo")

import numpy as np
from contextlib import ExitStack

# ---- dims ----
L = 2
D = 512
N = 8
H = 64
FF = 2048
F = 256
BQ = 8
BC = 32
TQ = 128
TC = 128
EPS = 1e-6
SCALE = 1.0 / 8.0

NCORES = 8
BCC = BC // NCORES
P = 128
KC = D // P          # 4 contraction chunks of 128
KC2 = KC // 2        # 2 DoubleRow pairs
MB = D // P
FFC = FF // P
FFC2 = FFC // 2
T1 = BCC * TC        # 512 tokens (layer-0, e-independent)
TB = 512             # token block
NBLK = BQ
T = NBLK * TB        # 4096 tokens (e-dependent)
TQALL = BQ * TQ      # 1024 query-memory tokens

SW = 64.0            # fp8 weight scale
SH = 8.0             # fp8 ffn-hidden scale

_BUILT = None


def build_program():
    import concourse.bass as bass
    import concourse.tile as tile
    import concourse.mybir as mybir
    from concourse import bacc

    F32 = mybir.dt.float32
    F32R = mybir.dt.float32r
    BF16 = mybir.dt.bfloat16
    F8 = mybir.dt.float8e4

    nc = bacc.Bacc("TRN2", target_bir_lowering=False, debug=False)
    tens = {}

    def din(name, shape, dt=F32):
        tens[name] = nc.dram_tensor(name, shape, dt, kind="ExternalInput")

    din("cfm_bf", [KC, P, T1], BF16)
    din("cfm_f8", [KC, P, T1], F8)
    din("qfm_bf", [KC, P, TQALL], BF16)
    din("qfm_f8", [KC, P, TQALL], F8)
    din("ones_bf", [1, P], BF16)
    for l in range(L):
        for pfx in ("sa", "ca"):
            din(f"{pfx}_wq_{l}", [KC, P, D], F8)
            din(f"{pfx}_wk_{l}", [KC, P, D], F8)
            din(f"{pfx}_wv_{l}", [KC, P, D], BF16)
            din(f"{pfx}_wo_{l}", [KC, P, D], BF16)
            din(f"{pfx}_bq_{l}", [MB, P])
            din(f"{pfx}_bk_{l}", [MB, P])
            din(f"{pfx}_bo_{l}", [MB, P])
            din(f"{pfx}_wqs_{l}", [MB, P])        # per-channel 1/scale
            din(f"{pfx}_wks_{l}", [MB, P])
            din(f"{pfx}_bv_{l}", [1, D], BF16)
        din(f"ffn_w1_{l}", [KC, P, FF], F8)
        din(f"ffn_w1s_{l}", [FFC, P])
        din(f"ffn_w2_{l}", [FFC, P, D], BF16)
    din("feat_wq", [KC, P, F], F32R)
    din("feat_wc", [KC, P, F], F32R)
    din("colsel", [P, 8, 8], F32R)
    din("colsel_bf", [P, 8, 8], BF16)
    din("rowsel_bf", [8, 8, P], BF16)
    din("selpair_bf", [8, 4, P], BF16)
    tens["out"] = nc.dram_tensor("out", [1, BQ * BCC], F32, kind="ExternalOutput")

    with tile.TileContext(nc) as tc, ExitStack() as ctx:
        with nc.allow_low_precision(reason="bf16/fp8 activations by design"):
            _emit(nc, tc, ctx, tens)
    nc.compile()
    return nc


def _emit(nc, tc, ctx, tens):
    import concourse.mybir as mybir

    F32 = mybir.dt.float32
    F32R = mybir.dt.float32r
    BF16 = mybir.dt.bfloat16
    F8 = mybir.dt.float8e4
    AF = mybir.ActivationFunctionType
    ALU = mybir.AluOpType
    DR = mybir.MatmulPerfMode.DoubleRow

    def r(ap):
        return ap.bitcast(F32R)

    # ---------------- pools ----------------
    const = ctx.enter_context(tc.tile_pool(name="const", bufs=1))
    persist = ctx.enter_context(tc.tile_pool(name="persist", bufs=1))
    stats_ch = ctx.enter_context(tc.tile_pool(name="stats_ch", bufs=1))
    ps = ctx.enter_context(tc.tile_pool(name="ps", bufs=2, space="PSUM"))
    ps_attn = ctx.enter_context(tc.tile_pool(name="ps_attn", bufs=3, space="PSUM"))
    ps_stats = ctx.enter_context(tc.tile_pool(name="ps_stats", bufs=1, space="PSUM"))
    ps_den = ctx.enter_context(tc.tile_pool(name="ps_den", bufs=1, space="PSUM"))

    # ---------------- constants ----------------
    eps_t = const.tile([P, 1], F32)
    nc.vector.memset(eps_t[:], EPS)
    colsel = const.tile([P, 8, 8], F32R)
    nc.sync.dma_start(colsel[:], tens["colsel"][:])
    colsel_bf = const.tile([P, 8, 8], BF16)
    nc.sync.dma_start(colsel_bf[:], tens["colsel_bf"][:])
    rowsel_bf = const.tile([8, 8, P], BF16)
    nc.sync.dma_start(rowsel_bf[:], tens["rowsel_bf"][:])
    selpair_bf = const.tile([8, 4, P], BF16)
    nc.sync.dma_start(selpair_bf[:], tens["selpair_bf"][:])
    ones_bf = const.tile([1, P], BF16)
    nc.sync.dma_start(ones_bf[:], tens["ones_bf"][:])

    # ---------------- persistent activations ----------------
    x_t = persist.tile([P, KC, T], BF16)       # residual stream (CA0 onward)
    x_f8 = persist.tile([P, KC, T], F8)        # fp8 shadow for projections
    q_f8 = persist.tile([P, KC, TQALL], F8)    # query memory (CA K proj input)
    nc.sync.dma_start(q_f8[:], tens["qfm_f8"].ap().rearrange("k p t -> p k t"))
    q_bf = persist.tile([P, KC, TQALL], BF16)  # query memory (CA V proj, pooling)
    nc.sync.dma_start(q_bf[:], tens["qfm_bf"].ap().rearrange("k p t -> p k t"))

    x0_stack = ExitStack()
    x0_pool = x0_stack.enter_context(tc.tile_pool(name="x0p", bufs=1))
    x0_t = x0_pool.tile([P, KC, T1], BF16)
    nc.sync.dma_start(x0_t[:], tens["cfm_bf"].ap().rearrange("k p t -> p k t"))
    x0_f8 = x0_pool.tile([P, KC, T1], F8)
    nc.sync.dma_start(x0_f8[:], tens["cfm_f8"].ap().rearrange("k p t -> p k t"))

    # ============ helpers ============

    def load_w(pool, name, kdim, ndim, dt):
        t = pool.tile([P, kdim, ndim], dt, tag=f"w_{name}")
        nc.sync.dma_start(t[:], tens[name].ap().rearrange("k p m -> p k m"))
        return t

    def load_pm(pool, name, cols=MB):
        t = pool.tile([P, cols], F32, tag=f"b_{name}")
        nc.sync.dma_start(t[:], tens[name].ap().rearrange("m p -> p m"))
        return t

    drain_ctr = [0]

    def drain(out, acc, scale=1.0, bias=None, relu=False):
        """PSUM -> SBUF with optional scale/bias/relu, alternating between
        the scalar and vector engines (gpsimd cannot read PSUM)."""
        eng = drain_ctr[0] % 2
        drain_ctr[0] += 1
        if eng == 0:
            func = AF.Relu if relu else (AF.Identity if bias is not None else AF.Copy)
            nc.scalar.activation(out, acc, func, scale=scale,
                                 bias=bias if bias is not None else 0.0)
        else:
            if relu:
                nc.vector.tensor_scalar(out, acc, scale, 0.0, ALU.mult, ALU.max)
            elif bias is not None:
                nc.vector.tensor_scalar(out, acc, scale, bias, ALU.mult, ALU.add)
            else:
                nc.vector.tensor_scalar_mul(out, acc, scale)

    def proj_f8(w_t, xf8_of, out_of_mb, nT, bias_t, scale_t):
        """out[mb] = (sum_k w[k,mb].T @ x[k]) * s_c + bias, fp8 DoubleRow
        with per-output-channel dequant scales."""
        for mb_i in range(MB):
            acc = ps.tile([P, TB], F32, tag="gemm")
            for kk in range(KC2):
                nc.tensor.matmul(
                    acc[:, :nT],
                    w_t[:, 2 * kk : 2 * kk + 2, mb_i * P : (mb_i + 1) * P],
                    xf8_of(kk),
                    start=(kk == 0), stop=(kk == KC2 - 1),
                    perf_mode=DR,
                )
            drain(out_of_mb(mb_i), acc[:, :nT],
                  scale=scale_t[:, mb_i : mb_i + 1],
                  bias=bias_t[:, mb_i : mb_i + 1])

    def proj_v(w_t, x_of_k, out_sb, bv_t):
        """Token-major bf16 V projection, one 128-token sub-block."""
        acc = ps.tile([P, TB], F32, tag="gemm")
        for k in range(KC):
            nc.tensor.matmul(
                acc[:, :D], x_of_k(k), w_t[:, k, :],
                start=(k == 0), stop=False,
            )
        nc.tensor.matmul(acc[:, :D], ones_bf[:], bv_t[:],
                         start=False, stop=True)
        drain(out_sb, acc[:, :D])

    def attn_block(pool, q_sb, k_of, v_of, o_sb, nsub):
        """MHA for one 512-token block; writes o_sb [P, MB, TB] bf16.

        q_sb [P, KC, TB] bf16. k_of(n, sub) -> [64, 128] bf16 keys;
        v_of(n, sub) -> [128, 64] bf16 values. nsub candidate sub-blocks
        share each 512-token q block; scores for all nsub subs of a head
        pack into one PSUM bank so exp/den work on [128, 512] slabs.
        """
        twid = TB // nsub
        e_all = pool.tile([P, N, TB], BF16, tag="exp")
        den_ps = ps_den.tile([8, TB], F32, tag="den")
        for n in range(N):
            hs = (n % 2) * H
            s_ps = ps_attn.tile([P, TB], F32, tag="attn")
            for sub in range(nsub):
                nc.tensor.matmul(
                    s_ps[:, sub * twid : (sub + 1) * twid],
                    k_of(n, sub),
                    q_sb[hs : hs + H, n // 2, sub * twid : (sub + 1) * twid],
                    start=(sub == 0), stop=(sub == nsub - 1),
                    skip_group_check=True,
                )
            nc.scalar.activation(e_all[:, n, :], s_ps[:], AF.Exp, scale=SCALE)
            nc.tensor.matmul(
                den_ps[:], colsel_bf[:, n, :], e_all[:, n, :],
                start=(n == 0), stop=(n == N - 1),
            )
        recip_f = pool.tile([8, TB], F32, tag="recipf")
        nc.vector.reciprocal_approx_fast(recip_f[:], den_ps[:])
        recip = pool.tile([8, TB], BF16, tag="recip")
        nc.gpsimd.tensor_scalar_mul(recip[:], recip_f[:], 1.0)
        for hp in range(4):
            av = ps_attn.tile([P, TB], F32, tag="attn")
            for sub in range(nsub):
                for j in range(2):
                    n = 2 * hp + j
                    nc.tensor.matmul(
                        av[j * H : (j + 1) * H, sub * twid : (sub + 1) * twid],
                        v_of(n, sub),
                        e_all[:, n, sub * twid : (sub + 1) * twid],
                        start=(sub == 0), stop=(sub == nsub - 1),
                        tile_position=(0, j * H),
                        skip_group_check=True,
                    )
            rb = ps_attn.tile([P, TB], F32, tag="attn")
            nc.tensor.matmul(rb[:], selpair_bf[:, hp, :], recip[:],
                             start=True, stop=True)
            rb_sb = pool.tile([P, TB], BF16, tag="rb")
            nc.scalar.copy(rb_sb[:], rb[:])
            nc.vector.tensor_tensor(o_sb[:, hp, :], av[:], rb_sb[:], ALU.mult)

    def oproj_residual_stats(pool, wo_t, bo_t, o_sb, x_out_of, x_res_of,
                             s1_ps, s2_ps, blk, nblk=NBLK):
        """x_out = wo.T @ o + bo + x_res (bf16); stats into row blk.

        The stats matmuls write the full [8, TB] psum (zero rows
        off-target), so only the very first matmul of the pass may use
        start=True.
        """
        for mb_i in range(MB):
            acc = ps.tile([P, TB], F32, tag="gemm")
            for k in range(KC):
                nc.tensor.matmul(
                    acc[:],
                    wo_t[:, k, mb_i * P : (mb_i + 1) * P],
                    o_sb[:, k, :],
                    start=(k == 0), stop=(k == KC - 1),
                )
            xo = x_out_of(mb_i)
            nc.vector.scalar_tensor_tensor(
                xo, acc[:], bo_t[:, mb_i : mb_i + 1],
                x_res_of(mb_i), ALU.add, ALU.add,
            )
            sq_t = pool.tile([P, TB], BF16, tag="sqc")
            nc.gpsimd.tensor_tensor(sq_t[:], xo, xo, ALU.mult)
            nc.tensor.matmul(s1_ps[:], colsel_bf[:, blk, :], xo,
                             start=(blk == 0 and mb_i == 0),
                             stop=(blk == nblk - 1 and mb_i == MB - 1))
            nc.tensor.matmul(s2_ps[:], colsel_bf[:, blk, :], sq_t[:],
                             start=(blk == 0 and mb_i == 0),
                             stop=(blk == nblk - 1 and mb_i == MB - 1))

    def ln_chain(s1, s2, nblk):
        mt = stats_ch.tile([8, TB], F32, tag="ln_m")
        nc.vector.tensor_scalar_mul(mt[:nblk], s1[:nblk], 1.0 / D)
        t1 = stats_ch.tile([8, TB], F32, tag="ln_u")
        nc.vector.tensor_tensor(t1[:nblk], mt[:nblk], mt[:nblk], ALU.mult)
        nc.vector.scalar_tensor_tensor(
            t1[:nblk], s2[:nblk], 1.0 / D, t1[:nblk], ALU.mult, ALU.subtract)
        sd = stats_ch.tile([8, TB], F32, tag="ln_sd")
        nc.scalar.activation(sd[:nblk], t1[:nblk], AF.Sqrt, bias=eps_t[:nblk, :])
        af = stats_ch.tile([8, TB], F32, tag="ln_af")
        scr = stats_ch.tile([8, TB], F32, tag="ln_scr")
        nc.vector.reciprocal_approx_accurate(af[:nblk], sd[:nblk], scr[:nblk])
        a_sb = stats_ch.tile([8, TB], BF16, tag="ln_a")
        nc.gpsimd.tensor_scalar_mul(a_sb[:nblk], af[:nblk], 1.0)
        c_sb = stats_ch.tile([8, TB], BF16, tag="ln_c")
        nc.vector.tensor_tensor(c_sb[:nblk], mt[:nblk], af[:nblk], ALU.mult)
        return a_sb, c_sb

    def ln_apply(pool, a_sb, c_sb, blk, x_of, xf8_of, nblk=NBLK):
        """x = x*a - c  (per-token a, c broadcast over partitions via PE).

        Writes bf16 x and (optionally) the fp8 shadow; work split between
        vector and gpsimd engines.
        """
        a_ps = ps.tile([P, TB], F32, tag="gemm")
        nc.tensor.matmul(a_ps[:], rowsel_bf[:nblk, blk, :], a_sb[:nblk, :],
                         start=True, stop=True)
        c_ps = ps.tile([P, TB], F32, tag="gemm")
        nc.tensor.matmul(c_ps[:], rowsel_bf[:nblk, blk, :], c_sb[:nblk, :],
                         start=True, stop=True)
        a_bc = pool.tile([P, TB], BF16, tag="lnab")
        nc.scalar.copy(a_bc[:], a_ps[:])
        c_bc = pool.tile([P, TB], BF16, tag="lncb")
        nc.scalar.copy(c_bc[:], c_ps[:])
        for mb_i in range(MB):
            e0 = nc.vector if mb_i % 2 == 0 else nc.gpsimd
            e1 = nc.gpsimd if mb_i % 2 == 0 else nc.vector
            tmp = pool.tile([P, TB], BF16, tag="lntmp")
            e0.tensor_tensor(tmp[:], x_of(mb_i), a_bc[:], ALU.mult)
            e0.tensor_tensor(x_of(mb_i), tmp[:], c_bc[:], ALU.subtract)
            if xf8_of is not None:
                e1.tensor_tensor(xf8_of(mb_i), tmp[:], c_bc[:], ALU.subtract)

    # ================= head: query side (no deps on x) =================
    with ExitStack() as hctx:
        hq = hctx.enter_context(tc.tile_pool(name="headq", bufs=1))
        fwq = load_w(hq, "feat_wq", KC, F, F32R)
        NF = F // P
        qp = persist.tile([P, KC, BQ], F32R)
        for k in range(KC):
            nc.vector.tensor_reduce(
                qp[:, k, :],
                q_bf[:, k, :].rearrange("p (e t) -> p e t", e=BQ)[:, :, 1:],
                mybir.AxisListType.X, ALU.add,
            )
        nc.vector.tensor_scalar_mul(qp[:], qp[:], 1.0 / (TQ - 1))
        qf = persist.tile([P, NF, BQ], F32R)
        qsq = persist.tile([P, NF, BQ], F32R)
        qq_ps = ps_attn.tile([8, TB], F32, tag="attn")
        for fb in range(NF):
            accq = ps.tile([P, TB], F32, tag="gemm")
            for k in range(KC):
                nc.tensor.matmul(accq[:, :BQ],
                                 r(fwq[:, k, fb * P : (fb + 1) * P]),
                                 r(qp[:, k, :]),
                                 start=(k == 0), stop=(k == KC - 1))
            nc.scalar.copy(qf[:, fb, :], accq[:, :BQ])
            nc.scalar.activation(qsq[:, fb, :], qf[:, fb, :], AF.Square)
            nc.tensor.matmul(qq_ps[:, :BQ], r(colsel[:, 0, :]), r(qsq[:, fb, :]),
                             start=(fb == 0), stop=(fb == NF - 1))
        rq = persist.tile([1, BQ], F32)
        t1q = hq.tile([1, BQ], F32)
        nc.vector.tensor_scalar_max(t1q[:], qq_ps[0:1, :BQ], 1e-12)
        t2q = hq.tile([1, BQ], F32)
        nc.scalar.activation(t2q[:], t1q[:], AF.Sqrt, bias=0.0)
        nc.vector.reciprocal(rq[:], t2q[:])
    cp = persist.tile([P, KC, BQ * BCC], F32R)

    # =========================================================
    for l in range(L):
        # ---------------- SA pass ----------------
        with ExitStack() as sctx:
            wp = sctx.enter_context(tc.tile_pool(name=f"saw{l}", bufs=1))
            tp = sctx.enter_context(tc.tile_pool(name=f"sat{l}", bufs=2))
            tp1 = sctx.enter_context(tc.tile_pool(name=f"sau{l}", bufs=2))
            wq = load_w(wp, f"sa_wq_{l}", KC, D, F8)
            wk = load_w(wp, f"sa_wk_{l}", KC, D, F8)
            wv = load_w(wp, f"sa_wv_{l}", KC, D, BF16)
            wo = load_w(wp, f"sa_wo_{l}", KC, D, BF16)
            bq = load_pm(wp, f"sa_bq_{l}")
            bk = load_pm(wp, f"sa_bk_{l}")
            bo = load_pm(wp, f"sa_bo_{l}")
            wqs = load_pm(wp, f"sa_wqs_{l}")
            wks = load_pm(wp, f"sa_wks_{l}")
            bv = wp.tile([1, D], BF16, tag="sabv")
            nc.sync.dma_start(bv[:], tens[f"sa_bv_{l}"][:])
            s1_ps = ps_stats.tile([8, TB], F32, tag="s1")
            s2_ps = ps_stats.tile([8, TB], F32, tag="s2")

            nblk = 1 if l == 0 else NBLK

            def xin_ap(m, blk):
                if l == 0:
                    return x0_t[:, m, :]
                return x_t[:, m, blk * TB : (blk + 1) * TB]

            def xf8_in(kk, blk):
                if l == 0:
                    return x0_f8[:, 2 * kk : 2 * kk + 2, :]
                return x_f8[:, 2 * kk : 2 * kk + 2, blk * TB : (blk + 1) * TB]

            def xf8_out(m, blk):
                if l == 0:
                    return x0_f8[:, m, :]
                return x_f8[:, m, blk * TB : (blk + 1) * TB]

            for blk in range(nblk):
                q_sb = tp.tile([P, KC, TB], BF16, tag="q")
                k_sb = tp.tile([P, KC, TB], BF16, tag="k")
                v_sb = tp.tile([P, BCC, D], BF16, tag="v")
                proj_f8(wq, lambda kk, blk=blk: xf8_in(kk, blk),
                        lambda m, q_sb=q_sb: q_sb[:, m, :], TB, bq, wqs)
                proj_f8(wk, lambda kk, blk=blk: xf8_in(kk, blk),
                        lambda m, k_sb=k_sb: k_sb[:, m, :], TB, bk, wks)
                for sub in range(BCC):
                    proj_v(wv,
                           lambda k, blk=blk, sub=sub: xin_ap(k, blk)[
                               :, sub * P : (sub + 1) * P],
                           v_sb[:, sub, :], bv)

                def k_of(n, sub, k_sb=k_sb):
                    hs = (n % 2) * H
                    return k_sb[hs : hs + H, n // 2, sub * P : (sub + 1) * P]

                def v_of(n, sub, v_sb=v_sb):
                    return v_sb[:, sub, n * H : (n + 1) * H]

                o_sb = tp.tile([P, MB, TB], BF16, tag="o")
                attn_block(tp, q_sb, k_of, v_of, o_sb, BCC)
                oproj_residual_stats(
                    tp1, wo, bo, o_sb,
                    lambda m, blk=blk: xin_ap(m, blk),
                    lambda m, blk=blk: xin_ap(m, blk),
                    s1_ps, s2_ps, blk, nblk=nblk,
                )
            a_sb, c_sb = ln_chain(s1_ps, s2_ps, nblk)
            for blk in range(nblk):
                ln_apply(tp1, a_sb, c_sb, blk,
                         lambda m, blk=blk: xin_ap(m, blk),
                         lambda m, blk=blk: xf8_out(m, blk), nblk=nblk)

        # ---------------- CA pass ----------------
        with ExitStack() as sctx:
            wp = sctx.enter_context(tc.tile_pool(name=f"caw{l}", bufs=1))
            tp = sctx.enter_context(tc.tile_pool(name=f"cat{l}", bufs=2))
            tp1 = sctx.enter_context(tc.tile_pool(name=f"cau{l}", bufs=2))
            wq = load_w(wp, f"ca_wq_{l}", KC, D, F8)
            wk = load_w(wp, f"ca_wk_{l}", KC, D, F8)
            wv = load_w(wp, f"ca_wv_{l}", KC, D, BF16)
            wo = load_w(wp, f"ca_wo_{l}", KC, D, BF16)
            bq = load_pm(wp, f"ca_bq_{l}")
            bk = load_pm(wp, f"ca_bk_{l}")
            bo = load_pm(wp, f"ca_bo_{l}")
            wqs = load_pm(wp, f"ca_wqs_{l}")
            wks = load_pm(wp, f"ca_wks_{l}")
            bv = wp.tile([1, D], BF16, tag="cabv")
            nc.sync.dma_start(bv[:], tens[f"ca_bv_{l}"][:])
            s1_ps = ps_stats.tile([8, TB], F32, tag="s1")
            s2_ps = ps_stats.tile([8, TB], F32, tag="s2")

            # K_ca^T [P, KC, TQALL] bf16 ; V_ca [P, BQ, D] bf16 (token-major)
            kca = wp.tile([P, KC, TQALL], BF16)
            for th in range(2):
                proj_f8(wk,
                        lambda kk, th=th: q_f8[:, 2 * kk : 2 * kk + 2,
                                               th * TB : (th + 1) * TB],
                        lambda m, th=th: kca[:, m, th * TB : (th + 1) * TB],
                        TB, bk, wks)
            vca = wp.tile([P, BQ, D], BF16)
            for e in range(BQ):
                proj_v(wv,
                       lambda k, e=e: q_bf[:, k, e * P : (e + 1) * P],
                       vca[:, e, :], bv)

            # L0: Q from x1 (e-independent) computed once
            if l == 0:
                q_sh = tp.tile([P, KC, TB], BF16, tag="q")
                proj_f8(wq, lambda kk: x0_f8[:, 2 * kk : 2 * kk + 2, :],
                        lambda m: q_sh[:, m, :], TB, bq, wqs)

            for e in range(NBLK):
                if l == 0:
                    q_sb = q_sh
                else:
                    q_sb = tp.tile([P, KC, TB], BF16, tag="q2")
                    proj_f8(wq,
                            lambda kk, e=e: x_f8[:, 2 * kk : 2 * kk + 2,
                                                 e * TB : (e + 1) * TB],
                            lambda m, q_sb=q_sb: q_sb[:, m, :], TB, bq, wqs)

                def k_of(n, sub, e=e):
                    hs = (n % 2) * H
                    return kca[hs : hs + H, n // 2, e * P : (e + 1) * P]

                def v_of(n, sub, e=e):
                    return vca[:, e, n * H : (n + 1) * H]

                o_sb = tp.tile([P, MB, TB], BF16, tag="o")
                attn_block(tp, q_sb, k_of, v_of, o_sb, 1)
                oproj_residual_stats(
                    tp1, wo, bo, o_sb,
                    lambda m, e=e: x_t[:, m, e * TB : (e + 1) * TB],
                    (lambda m: x0_t[:, m, :]) if l == 0 else
                    (lambda m, e=e: x_t[:, m, e * TB : (e + 1) * TB]),
                    s1_ps, s2_ps, e, NBLK,
                )
            a_sb, c_sb = ln_chain(s1_ps, s2_ps, NBLK)
            for blk in range(NBLK):
                ln_apply(tp1, a_sb, c_sb, blk,
                         lambda m, blk=blk: x_t[:, m, blk * TB : (blk + 1) * TB],
                         lambda m, blk=blk: x_f8[:, m, blk * TB : (blk + 1) * TB])
        if l == 0:
            x0_stack.close()

        # ---------------- FFN pass ----------------
        with ExitStack() as sctx:
            wp = sctx.enter_context(tc.tile_pool(name=f"fw{l}", bufs=1))
            tp1 = sctx.enter_context(tc.tile_pool(name=f"ft{l}", bufs=2))
            hp2 = sctx.enter_context(tc.tile_pool(name=f"fh{l}", bufs=2))
            w1 = load_w(wp, f"ffn_w1_{l}", KC, FF, F8)
            w1s = load_pm(wp, f"ffn_w1s_{l}", cols=FFC)
            w2 = load_w(wp, f"ffn_w2_{l}", FFC, D, BF16)
            s1_ps = ps_stats.tile([8, TB], F32, tag="s1")
            s2_ps = ps_stats.tile([8, TB], F32, tag="s2")
            last = l == L - 1

            for blk in range(NBLK):
                h_sb = hp2.tile([P, FFC, TB], BF16, tag="h")
                for mf in range(FFC):
                    acc = ps.tile([P, TB], F32, tag="gemm")
                    for kk in range(KC2):
                        nc.tensor.matmul(
                            acc[:],
                            w1[:, 2 * kk : 2 * kk + 2, mf * P : (mf + 1) * P],
                            x_f8[:, 2 * kk : 2 * kk + 2,
                                 blk * TB : (blk + 1) * TB],
                            start=(kk == 0), stop=(kk == KC2 - 1),
                            perf_mode=DR,
                        )
                    drain(h_sb[:, mf, :], acc[:],
                          scale=w1s[:, mf : mf + 1], relu=True)
                for mb_i in range(MB):
                    acc = ps.tile([P, TB], F32, tag="gemm")
                    for kf in range(FFC):
                        nc.tensor.matmul(
                            acc[:],
                            w2[:, kf, mb_i * P : (mb_i + 1) * P],
                            h_sb[:, kf, :],
                            start=(kf == 0), stop=(kf == FFC - 1),
                        )
                    xs = x_t[:, mb_i, blk * TB : (blk + 1) * TB]
                    nc.vector.tensor_tensor(xs, acc[:], xs, ALU.add)
                    sq_t = tp1.tile([P, TB], BF16, tag="sqc")
                    nc.gpsimd.tensor_tensor(sq_t[:], xs, xs, ALU.mult)
                    nc.tensor.matmul(s1_ps[:], colsel_bf[:, blk, :], xs,
                                     start=(blk == 0 and mb_i == 0),
                                     stop=(blk == NBLK - 1 and mb_i == MB - 1))
                    nc.tensor.matmul(s2_ps[:], colsel_bf[:, blk, :],
                                     sq_t[:],
                                     start=(blk == 0 and mb_i == 0),
                                     stop=(blk == NBLK - 1 and mb_i == MB - 1))
            a_sb, c_sb = ln_chain(s1_ps, s2_ps, NBLK)
            for blk in range(NBLK):
                ln_apply(tp1, a_sb, c_sb, blk,
                         lambda m, blk=blk: x_t[:, m, blk * TB : (blk + 1) * TB],
                         None if last else
                         (lambda m, blk=blk: x_f8[:, m,
                                                  blk * TB : (blk + 1) * TB]))
                if last:
                    # final LN (lnf) skipped: ln3 output has near-zero mean and
                    # variance v/(v+eps); lnf changes values by O(eps)=1e-6.
                    # candidate pooling per block, overlapped with later blocks
                    for k in range(KC):
                        nc.vector.tensor_reduce(
                            cp[:, k, blk * BCC : (blk + 1) * BCC],
                            x_t[:, k, blk * TB : (blk + 1) * TB].rearrange(
                                "p (c t) -> p c t", c=BCC)[:, :, 1:],
                            mybir.AxisListType.X, ALU.add,
                        )

    # ---------------- candidate feature head + cosine ----------------
    with ExitStack() as sctx:
        fp = sctx.enter_context(tc.tile_pool(name="fin", bufs=1))
        fwc = load_w(fp, "feat_wc", KC, F, F32R)
        NF = F // P  # 2
        NP = BQ * BCC  # 32

        nc.vector.tensor_scalar_mul(cp[:], cp[:], 1.0 / (TC - 1))
        cf = fp.tile([P, NF, NP], F32R)
        for fb in range(NF):
            accc = ps.tile([P, TB], F32, tag="gemm")
            for k in range(KC):
                nc.tensor.matmul(accc[:, :NP],
                                 r(fwc[:, k, fb * P : (fb + 1) * P]),
                                 r(cp[:, k, :]),
                                 start=(k == 0), stop=(k == KC - 1))
            nc.scalar.copy(cf[:, fb, :], accc[:, :NP])

        csq = fp.tile([P, NF, NP], F32R)
        z = fp.tile([P, NF, NP], F32R)
        cc_ps = ps_attn.tile([8, TB], F32, tag="attn")
        raw_ps = ps_den.tile([8, TB], F32, tag="den")
        for fb in range(NF):
            nc.scalar.activation(csq[:, fb, :], cf[:, fb, :], AF.Square)
            nc.vector.tensor_tensor(
                z[:, fb, :].rearrange("p (e c) -> p e c", e=BQ),
                cf[:, fb, :].rearrange("p (e c) -> p e c", e=BQ),
                qf[:, fb, :, None].to_broadcast((P, BQ, BCC)),
                ALU.mult,
            )
            nc.tensor.matmul(cc_ps[:, :NP], r(colsel[:, 0, :]), r(csq[:, fb, :]),
                             start=(fb == 0), stop=(fb == NF - 1))
            nc.tensor.matmul(raw_ps[:, :NP], r(colsel[:, 0, :]), r(z[:, fb, :]),
                             start=(fb == 0), stop=(fb == NF - 1))

        t1c = fp.tile([1, NP], F32)
        nc.vector.tensor_scalar_max(t1c[:], cc_ps[0:1, :NP], 1e-12)
        t2c = fp.tile([1, NP], F32)
        nc.scalar.activation(t2c[:], t1c[:], AF.Sqrt, bias=0.0)
        rc = fp.tile([1, NP], F32)
        nc.vector.reciprocal(rc[:], t2c[:])
        o1 = fp.tile([1, NP], F32)
        nc.vector.tensor_tensor(o1[:], raw_ps[0:1, :NP], rc[:], ALU.mult)
        o2 = fp.tile([1, NP], F32)
        nc.vector.tensor_tensor(
            o2[:].rearrange("p (e c) -> p e c", e=BQ),
            o1[:].rearrange("p (e c) -> p e c", e=BQ),
            rq[:, :, None].to_broadcast((1, BQ, BCC)),
            ALU.mult,
        )
        nc.sync.dma_start(tens["out"][:], o2[:])


# ================= host side =================

def _prep_inputs(inputs):
    """Build the per-core DRAM input maps from the full problem inputs."""
    import ml_dtypes

    f32 = np.float32
    bf16 = ml_dtypes.bfloat16
    f8 = ml_dtypes.float8_e4m3

    def to_f8(a):
        return np.clip(a, -240.0, 240.0).astype(f8)

    def quant_pc(w):
        """Per-output-channel fp8 quant of [K, ncol]; returns (w_f8, 1/s)."""
        s = 224.0 / np.clip(np.abs(w).max(0), 1e-9, None)
        return to_f8(w * s), (1.0 / s).astype(f32)

    gi = {k: np.asarray(v, f32) for k, v in inputs.items()}

    shared = {}
    q = gi["q"]  # [8, 128, 512]
    qfm = np.ascontiguousarray(q.reshape(TQALL, D).T.reshape(KC, P, TQALL))
    shared["qfm_bf"] = qfm.astype(bf16)
    shared["qfm_f8"] = to_f8(qfm)
    shared["ones_bf"] = np.ones((1, P), bf16)
    for l in range(L):
        for pfx in ("sa", "ca"):
            for wn in ("wq", "wk"):
                w = gi[f"{pfx}_{wn}"][l].reshape(D, D)
                wf8, sinv = quant_pc(w)
                shared[f"{pfx}_{wn}_{l}"] = np.ascontiguousarray(
                    wf8.reshape(KC, P, D))
                shared[f"{pfx}_{wn}s_{l}"] = np.ascontiguousarray(
                    sinv.reshape(MB, P))
            shared[f"{pfx}_wv_{l}"] = np.ascontiguousarray(
                gi[f"{pfx}_wv"][l].reshape(D, D).reshape(KC, P, D)).astype(bf16)
            wo = gi[f"{pfx}_wo"][l]  # [N, D, H]
            wo = np.ascontiguousarray(
                wo.transpose(0, 2, 1).reshape(D, D).reshape(KC, P, D))
            shared[f"{pfx}_wo_{l}"] = wo.astype(bf16)
            for bn in ("bq", "bk"):
                b = gi[f"{pfx}_{bn}"][l].reshape(D)
                shared[f"{pfx}_{bn}_{l}"] = np.ascontiguousarray(
                    b.reshape(MB, P))
            shared[f"{pfx}_bo_{l}"] = np.ascontiguousarray(
                gi[f"{pfx}_bo"][l].reshape(MB, P))
            shared[f"{pfx}_bv_{l}"] = np.ascontiguousarray(
                gi[f"{pfx}_bv"][l].reshape(1, D)).astype(bf16)
        w1f8, w1sinv = quant_pc(gi["ffn_w1"][l])
        shared[f"ffn_w1_{l}"] = np.ascontiguousarray(w1f8.reshape(KC, P, FF))
        shared[f"ffn_w1s_{l}"] = np.ascontiguousarray(w1sinv.reshape(FFC, P))
        shared[f"ffn_w2_{l}"] = np.ascontiguousarray(
            gi["ffn_w2"][l].reshape(FFC, P, D)).astype(bf16)
    shared["feat_wq"] = np.ascontiguousarray(gi["feat_wq"].reshape(KC, P, F))
    shared["feat_wc"] = np.ascontiguousarray(gi["feat_wc"].reshape(KC, P, F))

    colsel = np.zeros((P, 8, 8), f32)
    for j in range(8):
        colsel[:, j, j] = 1.0
    rowsel = np.zeros((8, 8, P), f32)
    for j in range(8):
        rowsel[j, j, :] = 1.0
    selpair = np.zeros((8, 4, P), f32)
    for hp in range(4):
        selpair[2 * hp, hp, :H] = 1.0
        selpair[2 * hp + 1, hp, H:] = 1.0
    shared["colsel"] = colsel
    shared["colsel_bf"] = colsel.astype(bf16)
    shared["rowsel_bf"] = rowsel.astype(bf16)
    shared["selpair_bf"] = selpair.astype(bf16)

    c = gi["c"]  # [32, 128, 512]
    in_maps = []
    for cc in range(NCORES):
        m = dict(shared)
        sl = c[cc * BCC : (cc + 1) * BCC].reshape(T1, D)
        cfm = np.ascontiguousarray(sl.T.reshape(KC, P, T1))
        m["cfm_bf"] = cfm.astype(bf16)
        m["cfm_f8"] = to_f8(cfm)
        in_maps.append(m)
    return in_maps


def kernel(**inputs):
    global _BUILT
    from concourse import bass_utils

    if _BUILT is None:
        _BUILT = build_program()
    nc = _BUILT
    in_maps = _prep_inputs(inputs)
    res = bass_utils.run_bass_kernel_spmd(nc, in_maps, list(range(NCORES)))
    outs = [res.results[i]["out"].reshape(BQ, BCC) for i in range(NCORES)]
    return np.concatenate(outs, axis=1).astype(np.float32)


# revision 12
# speedup vs baseline: 1.2644x; 1.2644x over previous
"""Trainium2 Bass kernel for nn_BloodhoundSub_12463995093069.

2-layer decoder with broadcast cross-attention -> cosine similarity [8, 32].

Sharding: candidates (BC=32) split 4 per core across 8 cores. Each core runs
the full decoder for its 4 candidates against all 8 query batches; the host
concatenates the per-core [8, 4] outputs along axis 1.

Layout: feature-major activations (features on partitions, tokens free).
Q/K/V and FFN projections run in fp8-e4m3 DoubleRow mode (2x PE rate,
weights pre-scaled by 64 on the host, rescaled in the PSUM drain); the
residual stream x is bf16 with an fp8 shadow written during LN-apply.
Attention scores for all candidate sub-blocks of a head share one PSUM bank
so exp/denominator work runs on [128,512] slabs. Cross-partition reductions
(LN stats, softmax denominators, partition broadcasts) are selector matmuls
on the PE. PSUM drains round-robin between the scalar and vector engines.
"""

import sys

if "/opt/trn_rl_repo" not in sys.path:
    sys.path.insert(0, "/opt/trn_rl_repo")

import numpy as np
from contextlib import ExitStack

# ---- dims ----
L = 2
D = 512
N = 8
H = 64
FF = 2048
F = 256
BQ = 8
BC = 32
TQ = 128
TC = 128
EPS = 1e-6
SCALE = 1.0 / 8.0

NCORES = 8
BCC = BC // NCORES
P = 128
KC = D // P          # 4 contraction chunks of 128
KC2 = KC // 2        # 2 DoubleRow pairs
MB = D // P
FFC = FF // P
FFC2 = FFC // 2
T1 = BCC * TC        # 512 tokens (layer-0, e-independent)
TB = 512             # token block
NBLK = BQ
T = NBLK * TB        # 4096 tokens (e-dependent)
TQALL = BQ * TQ      # 1024 query-memory tokens

SW = 64.0            # fp8 weight scale
SH = 8.0             # fp8 ffn-hidden scale

_BUILT = None


def build_program():
    import concourse.bass as bass
    import concourse.tile as tile
    import concourse.mybir as mybir
    from concourse import bacc

    F32 = mybir.dt.float32
    F32R = mybir.dt.float32r
    BF16 = mybir.dt.bfloat16
    F8 = mybir.dt.float8e4

    nc = bacc.Bacc("TRN2", target_bir_lowering=False, debug=False)
    tens = {}

    def din(name, shape, dt=F32):
        tens[name] = nc.dram_tensor(name, shape, dt, kind="ExternalInput")

    din("cfm_bf", [KC, P, T1], BF16)
    din("cfm_f8", [KC, P, T1], F8)
    din("qfm_bf", [KC, P, TQALL], BF16)
    din("qfm_f8", [KC, P, TQALL], F8)
    din("ones_bf", [1, P], BF16)
    for l in range(L):
        for pfx in ("sa", "ca"):
            din(f"{pfx}_wq_{l}", [KC, P, D], F8)
            din(f"{pfx}_wk_{l}", [KC, P, D], F8)
            din(f"{pfx}_wv_{l}", [KC, P, D], BF16)
            din(f"{pfx}_wo_{l}", [KC, P, D], BF16)
            din(f"{pfx}_bq_{l}", [MB, P])
            din(f"{pfx}_bk_{l}", [MB, P])
            din(f"{pfx}_bo_{l}", [MB, P])
            din(f"{pfx}_wqs_{l}", [MB, P])        # per-channel 1/scale
            din(f"{pfx}_wks_{l}", [MB, P])
            din(f"{pfx}_bv_{l}", [1, D], BF16)
        din(f"ffn_w1_{l}", [KC, P, FF], F8)
        din(f"ffn_w1s_{l}", [FFC, P])
        din(f"ffn_w2_{l}", [FFC, P, D], BF16)
    din("feat_wq", [KC, P, F], F32R)
    din("feat_wc", [KC, P, F], F32R)
    din("colsel", [P, 8, 8], F32R)
    din("colsel_bf", [P, 8, 8], BF16)
    din("rowsel_bf", [8, 8, P], BF16)
    din("selpair_bf", [8, 4, P], BF16)
    tens["out"] = nc.dram_tensor("out", [1, BQ * BCC], F32, kind="ExternalOutput")

    with tile.TileContext(nc) as tc, ExitStack() as ctx:
        with nc.allow_low_precision(reason="bf16/fp8 activations by design"):
            _emit(nc, tc, ctx, tens)
    nc.compile()
    return nc


def _emit(nc, tc, ctx, tens):
    import concourse.mybir as mybir

    F32 = mybir.dt.float32
    F32R = mybir.dt.float32r
    BF16 = mybir.dt.bfloat16
    F8 = mybir.dt.float8e4
    AF = mybir.ActivationFunctionType
    ALU = mybir.AluOpType
    DR = mybir.MatmulPerfMode.DoubleRow

    def r(ap):
        return ap.bitcast(F32R)

    # ---------------- pools ----------------
    const = ctx.enter_context(tc.tile_pool(name="const", bufs=1))
    persist = ctx.enter_context(tc.tile_pool(name="persist", bufs=1))
    stats_ch = ctx.enter_context(tc.tile_pool(name="stats_ch", bufs=1))
    ps = ctx.enter_context(tc.tile_pool(name="ps", bufs=3, space="PSUM"))
    ps_attn = ctx.enter_context(tc.tile_pool(name="ps_attn", bufs=3, space="PSUM"))
    ps_stats = ctx.enter_context(tc.tile_pool(name="ps_stats", bufs=1, space="PSUM"))

    # ---------------- constants ----------------
    eps_t = const.tile([P, 1], F32)
    nc.vector.memset(eps_t[:], EPS)
    colsel = const.tile([P, 8, 8], F32R)
    nc.sync.dma_start(colsel[:], tens["colsel"][:])
    colsel_bf = const.tile([P, 8, 8], BF16)
    nc.sync.dma_start(colsel_bf[:], tens["colsel_bf"][:])
    rowsel_bf = const.tile([8, 8, P], BF16)
    nc.sync.dma_start(rowsel_bf[:], tens["rowsel_bf"][:])
    selpair_bf = const.tile([8, 4, P], BF16)
    nc.sync.dma_start(selpair_bf[:], tens["selpair_bf"][:])
    ones_bf = const.tile([1, P], BF16)
    nc.sync.dma_start(ones_bf[:], tens["ones_bf"][:])

    # ---------------- persistent activations ----------------
    x_t = persist.tile([P, KC, T], BF16)       # residual stream (CA0 onward)
    x_f8 = persist.tile([P, KC, T], F8)        # fp8 shadow for projections
    q_f8 = persist.tile([P, KC, TQALL], F8)    # query memory (CA K proj input)
    nc.sync.dma_start(q_f8[:], tens["qfm_f8"].ap().rearrange("k p t -> p k t"))
    q_bf = persist.tile([P, KC, TQALL], BF16)  # query memory (CA V proj, pooling)
    nc.sync.dma_start(q_bf[:], tens["qfm_bf"].ap().rearrange("k p t -> p k t"))

    x0_stack = ExitStack()
    x0_pool = x0_stack.enter_context(tc.tile_pool(name="x0p", bufs=1))
    x0_t = x0_pool.tile([P, KC, T1], BF16)
    nc.sync.dma_start(x0_t[:], tens["cfm_bf"].ap().rearrange("k p t -> p k t"))
    x0_f8 = x0_pool.tile([P, KC, T1], F8)
    nc.sync.dma_start(x0_f8[:], tens["cfm_f8"].ap().rearrange("k p t -> p k t"))

    # ============ helpers ============

    def load_w(pool, name, kdim, ndim, dt):
        t = pool.tile([P, kdim, ndim], dt, tag=f"w_{name}")
        nc.sync.dma_start(t[:], tens[name].ap().rearrange("k p m -> p k m"))
        return t

    def load_pm(pool, name, cols=MB):
        t = pool.tile([P, cols], F32, tag=f"b_{name}")
        nc.sync.dma_start(t[:], tens[name].ap().rearrange("m p -> p m"))
        return t

    drain_ctr = [0]

    def drain(out, acc, scale=1.0, bias=None, relu=False):
        """PSUM -> SBUF with optional scale/bias/relu, alternating between
        the scalar and vector engines (gpsimd cannot read PSUM)."""
        eng = drain_ctr[0] % 2
        drain_ctr[0] += 1
        if eng == 0:
            func = AF.Relu if relu else (AF.Identity if bias is not None else AF.Copy)
            nc.scalar.activation(out, acc, func, scale=scale,
                                 bias=bias if bias is not None else 0.0)
        else:
            if relu:
                nc.vector.tensor_scalar(out, acc, scale, 0.0, ALU.mult, ALU.max)
            elif bias is not None:
                nc.vector.tensor_scalar(out, acc, scale, bias, ALU.mult, ALU.add)
            else:
                nc.vector.tensor_scalar_mul(out, acc, scale)

    def proj_f8(w_t, xf8_of, out_of_mb, nT, bias_t, scale_t):
        """out[mb] = (sum_k w[k,mb].T @ x[k]) * s_c + bias, fp8 DoubleRow
        with per-output-channel dequant scales."""
        for mb_i in range(MB):
            acc = ps.tile([P, TB], F32, tag="gemm")
            for kk in range(KC2):
                nc.tensor.matmul(
                    acc[:, :nT],
                    w_t[:, 2 * kk : 2 * kk + 2, mb_i * P : (mb_i + 1) * P],
                    xf8_of(kk),
                    start=(kk == 0), stop=(kk == KC2 - 1),
                    perf_mode=DR,
                )
            drain(out_of_mb(mb_i), acc[:, :nT],
                  scale=scale_t[:, mb_i : mb_i + 1],
                  bias=bias_t[:, mb_i : mb_i + 1])

    def proj_v(w_t, x_of_k, out_sb, bv_t):
        """Token-major bf16 V projection, one 128-token sub-block."""
        acc = ps.tile([P, TB], F32, tag="gemm")
        for k in range(KC):
            nc.tensor.matmul(
                acc[:, :D], x_of_k(k), w_t[:, k, :],
                start=(k == 0), stop=False,
            )
        nc.tensor.matmul(acc[:, :D], ones_bf[:], bv_t[:],
                         start=False, stop=True)
        drain(out_sb, acc[:, :D])

    def attn_block(pool, q_sb, k_of, v_of, o_sb, nsub):
        """MHA for one 512-token block; writes o_sb [P, MB, TB] bf16.

        q_sb [P, KC, TB] bf16. k_of(n, sub) -> [64, 128] bf16 keys;
        v_of(n, sub) -> [128, 64] bf16 values. nsub candidate sub-blocks
        share each 512-token q block; scores for all nsub subs of a head
        pack into one PSUM bank so exp/den work on [128, 512] slabs.
        """
        twid = TB // nsub
        e_all = pool.tile([P, N, TB], BF16, tag="exp")
        den_ps = ps_attn.tile([8, TB], F32, tag="attn")
        for n in range(N):
            hs = (n % 2) * H
            s_ps = ps_attn.tile([P, TB], F32, tag="attn")
            for sub in range(nsub):
                nc.tensor.matmul(
                    s_ps[:, sub * twid : (sub + 1) * twid],
                    k_of(n, sub),
                    q_sb[hs : hs + H, n // 2, sub * twid : (sub + 1) * twid],
                    start=(sub == 0), stop=(sub == nsub - 1),
                    skip_group_check=True,
                )
            nc.scalar.activation(e_all[:, n, :], s_ps[:], AF.Exp, scale=SCALE)
            nc.tensor.matmul(
                den_ps[:], colsel_bf[:, n, :], e_all[:, n, :],
                start=(n == 0), stop=(n == N - 1),
            )
        recip_f = pool.tile([8, TB], F32, tag="recipf")
        nc.vector.reciprocal_approx_fast(recip_f[:], den_ps[:])
        recip = pool.tile([8, TB], BF16, tag="recip")
        nc.scalar.copy(recip[:], recip_f[:])
        for hp in range(4):
            av = ps_attn.tile([P, TB], F32, tag="attn")
            for sub in range(nsub):
                for j in range(2):
                    n = 2 * hp + j
                    nc.tensor.matmul(
                        av[j * H : (j + 1) * H, sub * twid : (sub + 1) * twid],
                        v_of(n, sub),
                        e_all[:, n, sub * twid : (sub + 1) * twid],
                        start=(sub == 0), stop=(sub == nsub - 1),
                        tile_position=(0, j * H),
                        skip_group_check=True,
                    )
            rb = ps_attn.tile([P, TB], F32, tag="attn")
            nc.tensor.matmul(rb[:], selpair_bf[:, hp, :], recip[:],
                             start=True, stop=True)
            rb_sb = pool.tile([P, TB], BF16, tag="rb")
            nc.scalar.copy(rb_sb[:], rb[:])
            nc.vector.tensor_tensor(o_sb[:, hp, :], av[:], rb_sb[:], ALU.mult)

    def oproj_residual_stats(pool, wo_t, bo_t, o_sb, x_out_of, x_res_of,
                             s1_ps, s2_ps, blk, nblk=NBLK):
        """x_out = wo.T @ o + bo + x_res (bf16); stats into row blk.

        The stats matmuls write the full [8, TB] psum (zero rows
        off-target), so only the very first matmul of the pass may use
        start=True.
        """
        for mb_i in range(MB):
            acc = ps.tile([P, TB], F32, tag="gemm")
            for k in range(KC):
                nc.tensor.matmul(
                    acc[:],
                    wo_t[:, k, mb_i * P : (mb_i + 1) * P],
                    o_sb[:, k, :],
                    start=(k == 0), stop=(k == KC - 1),
                )
            xo = x_out_of(mb_i)
            nc.vector.scalar_tensor_tensor(
                xo, acc[:], bo_t[:, mb_i : mb_i + 1],
                x_res_of(mb_i), ALU.add, ALU.add,
            )
            sq_t = pool.tile([P, TB], BF16, tag="sqc")
            nc.scalar.activation(sq_t[:], xo, AF.Square)
            nc.tensor.matmul(s1_ps[:], colsel_bf[:, blk, :], xo,
                             start=(blk == 0 and mb_i == 0),
                             stop=(blk == nblk - 1 and mb_i == MB - 1))
            nc.tensor.matmul(s2_ps[:], colsel_bf[:, blk, :], sq_t[:],
                             start=(blk == 0 and mb_i == 0),
                             stop=(blk == nblk - 1 and mb_i == MB - 1))

    def ln_chain(s1, s2, nblk):
        mt = stats_ch.tile([8, TB], F32, tag="ln_m")
        nc.vector.tensor_scalar_mul(mt[:nblk], s1[:nblk], 1.0 / D)
        t1 = stats_ch.tile([8, TB], F32, tag="ln_u")
        nc.vector.tensor_tensor(t1[:nblk], mt[:nblk], mt[:nblk], ALU.mult)
        nc.vector.scalar_tensor_tensor(
            t1[:nblk], s2[:nblk], 1.0 / D, t1[:nblk], ALU.mult, ALU.subtract)
        sd = stats_ch.tile([8, TB], F32, tag="ln_sd")
        nc.scalar.activation(sd[:nblk], t1[:nblk], AF.Sqrt, bias=eps_t[:nblk, :])
        af = stats_ch.tile([8, TB], F32, tag="ln_af")
        scr = stats_ch.tile([8, TB], F32, tag="ln_scr")
        nc.vector.reciprocal_approx_accurate(af[:nblk], sd[:nblk], scr[:nblk])
        a_sb = stats_ch.tile([8, TB], BF16, tag="ln_a")
        nc.scalar.copy(a_sb[:nblk], af[:nblk])
        c_sb = stats_ch.tile([8, TB], BF16, tag="ln_c")
        nc.vector.tensor_tensor(c_sb[:nblk], mt[:nblk], af[:nblk], ALU.mult)
        return a_sb, c_sb

    def ln_apply(pool, a_sb, c_sb, blk, x_of, xf8_of, nblk=NBLK):
        """x = x*a - c  (per-token a, c broadcast over partitions via PE).

        Writes bf16 x and (optionally) the fp8 shadow; work split between
        vector and gpsimd engines.
        """
        a_ps = ps.tile([P, TB], F32, tag="gemm")
        nc.tensor.matmul(a_ps[:], rowsel_bf[:nblk, blk, :], a_sb[:nblk, :],
                         start=True, stop=True)
        c_ps = ps.tile([P, TB], F32, tag="gemm")
        nc.tensor.matmul(c_ps[:], rowsel_bf[:nblk, blk, :], c_sb[:nblk, :],
                         start=True, stop=True)
        for mb_i in range(MB):
            tmp = pool.tile([P, TB], F32, tag="lntmp")
            nc.vector.tensor_tensor(tmp[:], x_of(mb_i), a_ps[:], ALU.mult)
            nc.vector.tensor_tensor(x_of(mb_i), tmp[:], c_ps[:], ALU.subtract)
            if xf8_of is not None:
                nc.scalar.copy(xf8_of(mb_i), x_of(mb_i))

    # ================= head: query side (no deps on x) =================
    with ExitStack() as hctx:
        hq = hctx.enter_context(tc.tile_pool(name="headq", bufs=1))
        fwq = load_w(hq, "feat_wq", KC, F, F32R)
        NF = F // P
        qp = persist.tile([P, KC, BQ], F32R)
        for k in range(KC):
            nc.vector.tensor_reduce(
                qp[:, k, :],
                q_bf[:, k, :].rearrange("p (e t) -> p e t", e=BQ)[:, :, 1:],
                mybir.AxisListType.X, ALU.add,
            )
        nc.vector.tensor_scalar_mul(qp[:], qp[:], 1.0 / (TQ - 1))
        qf = persist.tile([P, NF, BQ], F32R)
        qsq = persist.tile([P, NF, BQ], F32R)
        qq_ps = ps_attn.tile([8, TB], F32, tag="attn")
        for fb in range(NF):
            accq = ps.tile([P, TB], F32, tag="gemm")
            for k in range(KC):
                nc.tensor.matmul(accq[:, :BQ],
                                 r(fwq[:, k, fb * P : (fb + 1) * P]),
                                 r(qp[:, k, :]),
                                 start=(k == 0), stop=(k == KC - 1))
            nc.scalar.copy(qf[:, fb, :], accq[:, :BQ])
            nc.scalar.activation(qsq[:, fb, :], qf[:, fb, :], AF.Square)
            nc.tensor.matmul(qq_ps[:, :BQ], r(colsel[:, 0, :]), r(qsq[:, fb, :]),
                             start=(fb == 0), stop=(fb == NF - 1))
        rq = persist.tile([1, BQ], F32)
        t1q = hq.tile([1, BQ], F32)
        nc.vector.tensor_scalar_max(t1q[:], qq_ps[0:1, :BQ], 1e-12)
        t2q = hq.tile([1, BQ], F32)
        nc.scalar.activation(t2q[:], t1q[:], AF.Sqrt, bias=0.0)
        nc.vector.reciprocal(rq[:], t2q[:])
    cp = persist.tile([P, KC, BQ * BCC], F32R)

    # =========================================================
    for l in range(L):
        # ---------------- SA pass ----------------
        with ExitStack() as sctx:
            wp = sctx.enter_context(tc.tile_pool(name=f"saw{l}", bufs=1))
            tp = sctx.enter_context(tc.tile_pool(name=f"sat{l}", bufs=2))
            tp1 = sctx.enter_context(tc.tile_pool(name=f"sau{l}", bufs=2))
            wq = load_w(wp, f"sa_wq_{l}", KC, D, F8)
            wk = load_w(wp, f"sa_wk_{l}", KC, D, F8)
            wv = load_w(wp, f"sa_wv_{l}", KC, D, BF16)
            wo = load_w(wp, f"sa_wo_{l}", KC, D, BF16)
            bq = load_pm(wp, f"sa_bq_{l}")
            bk = load_pm(wp, f"sa_bk_{l}")
            bo = load_pm(wp, f"sa_bo_{l}")
            wqs = load_pm(wp, f"sa_wqs_{l}")
            wks = load_pm(wp, f"sa_wks_{l}")
            bv = wp.tile([1, D], BF16, tag="sabv")
            nc.sync.dma_start(bv[:], tens[f"sa_bv_{l}"][:])
            s1_ps = ps_stats.tile([8, TB], F32, tag="s1")
            s2_ps = ps_stats.tile([8, TB], F32, tag="s2")

            nblk = 1 if l == 0 else NBLK

            def xin_ap(m, blk):
                if l == 0:
                    return x0_t[:, m, :]
                return x_t[:, m, blk * TB : (blk + 1) * TB]

            def xf8_in(kk, blk):
                if l == 0:
                    return x0_f8[:, 2 * kk : 2 * kk + 2, :]
                return x_f8[:, 2 * kk : 2 * kk + 2, blk * TB : (blk + 1) * TB]

            def xf8_out(m, blk):
                if l == 0:
                    return x0_f8[:, m, :]
                return x_f8[:, m, blk * TB : (blk + 1) * TB]

            for blk in range(nblk):
                q_sb = tp.tile([P, KC, TB], BF16, tag="q")
                k_sb = tp.tile([P, KC, TB], BF16, tag="k")
                v_sb = tp.tile([P, BCC, D], BF16, tag="v")
                proj_f8(wq, lambda kk, blk=blk: xf8_in(kk, blk),
                        lambda m, q_sb=q_sb: q_sb[:, m, :], TB, bq, wqs)
                proj_f8(wk, lambda kk, blk=blk: xf8_in(kk, blk),
                        lambda m, k_sb=k_sb: k_sb[:, m, :], TB, bk, wks)
                for sub in range(BCC):
                    proj_v(wv,
                           lambda k, blk=blk, sub=sub: xin_ap(k, blk)[
                               :, sub * P : (sub + 1) * P],
                           v_sb[:, sub, :], bv)

                def k_of(n, sub, k_sb=k_sb):
                    hs = (n % 2) * H
                    return k_sb[hs : hs + H, n // 2, sub * P : (sub + 1) * P]

                def v_of(n, sub, v_sb=v_sb):
                    return v_sb[:, sub, n * H : (n + 1) * H]

                o_sb = tp.tile([P, MB, TB], BF16, tag="o")
                attn_block(tp, q_sb, k_of, v_of, o_sb, BCC)
                oproj_residual_stats(
                    tp1, wo, bo, o_sb,
                    lambda m, blk=blk: xin_ap(m, blk),
                    lambda m, blk=blk: xin_ap(m, blk),
                    s1_ps, s2_ps, blk, nblk=nblk,
                )
            a_sb, c_sb = ln_chain(s1_ps, s2_ps, nblk)
            for blk in range(nblk):
                ln_apply(tp1, a_sb, c_sb, blk,
                         lambda m, blk=blk: xin_ap(m, blk),
                         lambda m, blk=blk: xf8_out(m, blk), nblk=nblk)

        # ---------------- CA pass ----------------
        with ExitStack() as sctx:
            wp = sctx.enter_context(tc.tile_pool(name=f"caw{l}", bufs=1))
            tp = sctx.enter_context(tc.tile_pool(name=f"cat{l}", bufs=2))
            tp1 = sctx.enter_context(tc.tile_pool(name=f"cau{l}", bufs=2))
            wq = load_w(wp, f"ca_wq_{l}", KC, D, F8)
            wk = load_w(wp, f"ca_wk_{l}", KC, D, F8)
            wv = load_w(wp, f"ca_wv_{l}", KC, D, BF16)
            wo = load_w(wp, f"ca_wo_{l}", KC, D, BF16)
            bq = load_pm(wp, f"ca_bq_{l}")
            bk = load_pm(wp, f"ca_bk_{l}")
            bo = load_pm(wp, f"ca_bo_{l}")
            wqs = load_pm(wp, f"ca_wqs_{l}")
            wks = load_pm(wp, f"ca_wks_{l}")
            bv = wp.tile([1, D], BF16, tag="cabv")
            nc.sync.dma_start(bv[:], tens[f"ca_bv_{l}"][:])
            s1_ps = ps_stats.tile([8, TB], F32, tag="s1")
            s2_ps = ps_stats.tile([8, TB], F32, tag="s2")

            # K_ca^T [P, KC, TQALL] bf16 ; V_ca [P, BQ, D] bf16 (token-major)
            kca = wp.tile([P, KC, TQALL], BF16)
            for th in range(2):
                proj_f8(wk,
                        lambda kk, th=th: q_f8[:, 2 * kk : 2 * kk + 2,
                                               th * TB : (th + 1) * TB],
                        lambda m, th=th: kca[:, m, th * TB : (th + 1) * TB],
                        TB, bk, wks)
            vca = wp.tile([P, BQ, D], BF16)
            for e in range(BQ):
                proj_v(wv,
                       lambda k, e=e: q_bf[:, k, e * P : (e + 1) * P],
                       vca[:, e, :], bv)

            # L0: Q from x1 (e-independent) computed once
            if l == 0:
                q_sh = tp.tile([P, KC, TB], BF16, tag="q")
                proj_f8(wq, lambda kk: x0_f8[:, 2 * kk : 2 * kk + 2, :],
                        lambda m: q_sh[:, m, :], TB, bq, wqs)

            for e in range(NBLK):
                if l == 0:
                    q_sb = q_sh
                else:
                    q_sb = tp.tile([P, KC, TB], BF16, tag="q2")
                    proj_f8(wq,
                            lambda kk, e=e: x_f8[:, 2 * kk : 2 * kk + 2,
                                                 e * TB : (e + 1) * TB],
                            lambda m, q_sb=q_sb: q_sb[:, m, :], TB, bq, wqs)

                def k_of(n, sub, e=e):
                    hs = (n % 2) * H
                    return kca[hs : hs + H, n // 2, e * P : (e + 1) * P]

                def v_of(n, sub, e=e):
                    return vca[:, e, n * H : (n + 1) * H]

                o_sb = tp.tile([P, MB, TB], BF16, tag="o")
                attn_block(tp, q_sb, k_of, v_of, o_sb, 1)
                oproj_residual_stats(
                    tp1, wo, bo, o_sb,
                    lambda m, e=e: x_t[:, m, e * TB : (e + 1) * TB],
                    (lambda m: x0_t[:, m, :]) if l == 0 else
                    (lambda m, e=e: x_t[:, m, e * TB : (e + 1) * TB]),
                    s1_ps, s2_ps, e, NBLK,
                )
            a_sb, c_sb = ln_chain(s1_ps, s2_ps, NBLK)
            for blk in range(NBLK):
                ln_apply(tp1, a_sb, c_sb, blk,
                         lambda m, blk=blk: x_t[:, m, blk * TB : (blk + 1) * TB],
                         lambda m, blk=blk: x_f8[:, m, blk * TB : (blk + 1) * TB])
        if l == 0:
            x0_stack.close()

        # ---------------- FFN pass ----------------
        with ExitStack() as sctx:
            wp = sctx.enter_context(tc.tile_pool(name=f"fw{l}", bufs=1))
            tp1 = sctx.enter_context(tc.tile_pool(name=f"ft{l}", bufs=2))
            hp2 = sctx.enter_context(tc.tile_pool(name=f"fh{l}", bufs=2))
            w1 = load_w(wp, f"ffn_w1_{l}", KC, FF, F8)
            w1s = load_pm(wp, f"ffn_w1s_{l}", cols=FFC)
            w2 = load_w(wp, f"ffn_w2_{l}", FFC, D, BF16)
            s1_ps = ps_stats.tile([8, TB], F32, tag="s1")
            s2_ps = ps_stats.tile([8, TB], F32, tag="s2")
            last = l == L - 1

            for blk in range(NBLK):
                h_sb = hp2.tile([P, FFC, TB], BF16, tag="h")
                for mf in range(FFC):
                    acc = ps.tile([P, TB], F32, tag="gemm")
                    for kk in range(KC2):
                        nc.tensor.matmul(
                            acc[:],
                            w1[:, 2 * kk : 2 * kk + 2, mf * P : (mf + 1) * P],
                            x_f8[:, 2 * kk : 2 * kk + 2,
                                 blk * TB : (blk + 1) * TB],
                            start=(kk == 0), stop=(kk == KC2 - 1),
                            perf_mode=DR,
                        )
                    drain(h_sb[:, mf, :], acc[:],
                          scale=w1s[:, mf : mf + 1], relu=True)
                for mb_i in range(MB):
                    acc = ps.tile([P, TB], F32, tag="gemm")
                    for kf in range(FFC):
                        nc.tensor.matmul(
                            acc[:],
                            w2[:, kf, mb_i * P : (mb_i + 1) * P],
                            h_sb[:, kf, :],
                            start=(kf == 0), stop=(kf == FFC - 1),
                        )
                    xs = x_t[:, mb_i, blk * TB : (blk + 1) * TB]
                    nc.vector.tensor_tensor(xs, acc[:], xs, ALU.add)
                    sq_t = tp1.tile([P, TB], BF16, tag="sqc")
                    nc.scalar.activation(sq_t[:], xs, AF.Square)
                    nc.tensor.matmul(s1_ps[:], colsel_bf[:, blk, :], xs,
                                     start=(blk == 0 and mb_i == 0),
                                     stop=(blk == NBLK - 1 and mb_i == MB - 1))
                    nc.tensor.matmul(s2_ps[:], colsel_bf[:, blk, :],
                                     sq_t[:],
                                     start=(blk == 0 and mb_i == 0),
                                     stop=(blk == NBLK - 1 and mb_i == MB - 1))
            a_sb, c_sb = ln_chain(s1_ps, s2_ps, NBLK)
            for blk in range(NBLK):
                ln_apply(tp1, a_sb, c_sb, blk,
                         lambda m, blk=blk: x_t[:, m, blk * TB : (blk + 1) * TB],
                         None if last else
                         (lambda m, blk=blk: x_f8[:, m,
                                                  blk * TB : (blk + 1) * TB]))
                if last:
                    # final LN (lnf) skipped: ln3 output has near-zero mean and
                    # variance v/(v+eps); lnf changes values by O(eps)=1e-6.
                    # candidate pooling per block, overlapped with later blocks
                    for k in range(KC):
                        nc.vector.tensor_reduce(
                            cp[:, k, blk * BCC : (blk + 1) * BCC],
                            x_t[:, k, blk * TB : (blk + 1) * TB].rearrange(
                                "p (c t) -> p c t", c=BCC)[:, :, 1:],
                            mybir.AxisListType.X, ALU.add,
                        )

    # ---------------- candidate feature head + cosine ----------------
    with ExitStack() as sctx:
        fp = sctx.enter_context(tc.tile_pool(name="fin", bufs=1))
        fwc = load_w(fp, "feat_wc", KC, F, F32R)
        NF = F // P  # 2
        NP = BQ * BCC  # 32

        nc.vector.tensor_scalar_mul(cp[:], cp[:], 1.0 / (TC - 1))
        cf = fp.tile([P, NF, NP], F32R)
        for fb in range(NF):
            accc = ps.tile([P, TB], F32, tag="gemm")
            for k in range(KC):
                nc.tensor.matmul(accc[:, :NP],
                                 r(fwc[:, k, fb * P : (fb + 1) * P]),
                                 r(cp[:, k, :]),
                                 start=(k == 0), stop=(k == KC - 1))
            nc.scalar.copy(cf[:, fb, :], accc[:, :NP])

        csq = fp.tile([P, NF, NP], F32R)
        z = fp.tile([P, NF, NP], F32R)
        cc_ps = ps_attn.tile([8, TB], F32, tag="attn")
        raw_ps = ps_attn.tile([8, TB], F32, tag="attn")
        for fb in range(NF):
            nc.scalar.activation(csq[:, fb, :], cf[:, fb, :], AF.Square)
            nc.vector.tensor_tensor(
                z[:, fb, :].rearrange("p (e c) -> p e c", e=BQ),
                cf[:, fb, :].rearrange("p (e c) -> p e c", e=BQ),
                qf[:, fb, :, None].to_broadcast((P, BQ, BCC)),
                ALU.mult,
            )
            nc.tensor.matmul(cc_ps[:, :NP], r(colsel[:, 0, :]), r(csq[:, fb, :]),
                             start=(fb == 0), stop=(fb == NF - 1))
            nc.tensor.matmul(raw_ps[:, :NP], r(colsel[:, 0, :]), r(z[:, fb, :]),
                             start=(fb == 0), stop=(fb == NF - 1))

        t1c = fp.tile([1, NP], F32)
        nc.vector.tensor_scalar_max(t1c[:], cc_ps[0:1, :NP], 1e-12)
        t2c = fp.tile([1, NP], F32)
        nc.scalar.activation(t2c[:], t1c[:], AF.Sqrt, bias=0.0)
        rc = fp.tile([1, NP], F32)
        nc.vector.reciprocal(rc[:], t2c[:])
        o1 = fp.tile([1, NP], F32)
        nc.vector.tensor_tensor(o1[:], raw_ps[0:1, :NP], rc[:], ALU.mult)
        o2 = fp.tile([1, NP], F32)
        nc.vector.tensor_tensor(
            o2[:].rearrange("p (e c) -> p e c", e=BQ),
            o1[:].rearrange("p (e c) -> p e c", e=BQ),
            rq[:, :, None].to_broadcast((1, BQ, BCC)),
            ALU.mult,
        )
        nc.sync.dma_start(tens["out"][:], o2[:])


# ================= host side =================

def _prep_inputs(inputs):
    """Build the per-core DRAM input maps from the full problem inputs."""
    import ml_dtypes

    f32 = np.float32
    bf16 = ml_dtypes.bfloat16
    f8 = ml_dtypes.float8_e4m3

    def to_f8(a):
        return np.clip(a, -240.0, 240.0).astype(f8)

    def quant_pc(w):
        """Per-output-channel fp8 quant of [K, ncol]; returns (w_f8, 1/s)."""
        s = 224.0 / np.clip(np.abs(w).max(0), 1e-9, None)
        return to_f8(w * s), (1.0 / s).astype(f32)

    gi = {k: np.asarray(v, f32) for k, v in inputs.items()}

    shared = {}
    q = gi["q"]  # [8, 128, 512]
    qfm = np.ascontiguousarray(q.reshape(TQALL, D).T.reshape(KC, P, TQALL))
    shared["qfm_bf"] = qfm.astype(bf16)
    shared["qfm_f8"] = to_f8(qfm)
    shared["ones_bf"] = np.ones((1, P), bf16)
    for l in range(L):
        for pfx in ("sa", "ca"):
            for wn in ("wq", "wk"):
                w = gi[f"{pfx}_{wn}"][l].reshape(D, D)
                wf8, sinv = quant_pc(w)
                shared[f"{pfx}_{wn}_{l}"] = np.ascontiguousarray(
                    wf8.reshape(KC, P, D))
                shared[f"{pfx}_{wn}s_{l}"] = np.ascontiguousarray(
                    sinv.reshape(MB, P))
            shared[f"{pfx}_wv_{l}"] = np.ascontiguousarray(
                gi[f"{pfx}_wv"][l].reshape(D, D).reshape(KC, P, D)).astype(bf16)
            wo = gi[f"{pfx}_wo"][l]  # [N, D, H]
            wo = np.ascontiguousarray(
                wo.transpose(0, 2, 1).reshape(D, D).reshape(KC, P, D))
            shared[f"{pfx}_wo_{l}"] = wo.astype(bf16)
            for bn in ("bq", "bk"):
                b = gi[f"{pfx}_{bn}"][l].reshape(D)
                shared[f"{pfx}_{bn}_{l}"] = np.ascontiguousarray(
                    b.reshape(MB, P))
            shared[f"{pfx}_bo_{l}"] = np.ascontiguousarray(
                gi[f"{pfx}_bo"][l].reshape(MB, P))
            shared[f"{pfx}_bv_{l}"] = np.ascontiguousarray(
                gi[f"{pfx}_bv"][l].reshape(1, D)).astype(bf16)
        w1f8, w1sinv = quant_pc(gi["ffn_w1"][l])
        shared[f"ffn_w1_{l}"] = np.ascontiguousarray(w1f8.reshape(KC, P, FF))
        shared[f"ffn_w1s_{l}"] = np.ascontiguousarray(w1sinv.reshape(FFC, P))
        shared[f"ffn_w2_{l}"] = np.ascontiguousarray(
            gi["ffn_w2"][l].reshape(FFC, P, D)).astype(bf16)
    shared["feat_wq"] = np.ascontiguousarray(gi["feat_wq"].reshape(KC, P, F))
    shared["feat_wc"] = np.ascontiguousarray(gi["feat_wc"].reshape(KC, P, F))

    colsel = np.zeros((P, 8, 8), f32)
    for j in range(8):
        colsel[:, j, j] = 1.0
    rowsel = np.zeros((8, 8, P), f32)
    for j in range(8):
        rowsel[j, j, :] = 1.0
    selpair = np.zeros((8, 4, P), f32)
    for hp in range(4):
        selpair[2 * hp, hp, :H] = 1.0
        selpair[2 * hp + 1, hp, H:] = 1.0
    shared["colsel"] = colsel
    shared["colsel_bf"] = colsel.astype(bf16)
    shared["rowsel_bf"] = rowsel.astype(bf16)
    shared["selpair_bf"] = selpair.astype(bf16)

    c = gi["c"]  # [32, 128, 512]
    in_maps = []
    for cc in range(NCORES):
        m = dict(shared)
        sl = c[cc * BCC : (cc + 1) * BCC].reshape(T1, D)
        cfm = np.ascontiguousarray(sl.T.reshape(KC, P, T1))
        m["cfm_bf"] = cfm.astype(bf16)
        m["cfm_f8"] = to_f8(cfm)
        in_maps.append(m)
    return in_maps


def kernel(**inputs):
    global _BUILT
    from concourse import bass_utils

    if _BUILT is None:
        _BUILT = build_program()
    nc = _BUILT
    in_maps = _prep_inputs(inputs)
    res = bass_utils.run_bass_kernel_spmd(nc, in_maps, list(range(NCORES)))
    outs = [res.results[i]["out"].reshape(BQ, BCC) for i in range(NCORES)]
    return np.concatenate(outs, axis=1).astype(np.float32)
